# revision 12
# baseline (speedup 1.0000x reference)
"""AERGCN (2-layer R-GCN + bilinear attention pool) on 8 TRN2 NeuronCores.

Sharding: pair-hybrid. Cores are paired (2p, 2p+1); pair p owns batches
A=2p, B=2p+1. Within a pair the 41 relations split 20/20 (even core: rels
0-19, odd: 20-39) and relation 40 is computed by BOTH cores at half weight
(the 0.5 is folded into the relation-softmax exp bias as +ln 0.5), keeping
the graph fully SPMD-symmetric. Each layer runs one stream of 42 (rel,
batch) combos strictly alternating A/B (A leads by 2) so each weight tile
is DMA'd once and consumed by both batches back-to-back, keeping weight
demand at a steady 1 tile / 2 combos. Per layer, ONE pairwise AllReduce
([2,S,769] bf16: full-A | full-B payloads) reconstitutes the relation
softmax for both batches in a single collective (fixed CC cost ~11us paid
once per layer, not twice). 1/denom is precomputed on host, so combos
issue no GpSimd work and CC triggers can't head-of-line-block the pipe.
After layer 2 each core runs the attention pool for its own batch
(selected from the AllReduce output by a data-driven mask to stay SPMD).

Matmuls in bf16 (f32 PSUM). Per-combo pipeline:
  hidden = h @ [W_r | W_r @ score_w]          (12 accumulating matmuls)
  logun = adj @ u                             (1 matmul, N=1, lhsT=adjT)
  e = exp(logun*rec + bias); scr = e*rec      (rec = 1/denom from host)
  payload += scr * (adj @ hidden)             (2 matmuls N=384; DVE)
"""

import os
import sys

# The Bass NEFF executes through the axon PJRT backend; if the caller pinned
# jax to cpu before we ever import jax, lift the pin so axon devices resolve.
if "jax" not in sys.modules and os.environ.get("JAX_PLATFORMS") == "cpu":
    os.environ["JAX_PLATFORMS"] = ""

import numpy as np
import ml_dtypes

bf16 = ml_dtypes.bfloat16

B, S, F, R, NL = 8, 128, 768, 41, 2
NH, HD, EMB = 8, 96, 768
NCORES, IC = 8, 6
FE = F + 1      # 769: W with appended u column
RSLOT = 21      # 20 private relations + shared relation 40 (half weight)
LEAD = 2        # batch A runs this many relations ahead of batch B

_CACHE = {}


def _build_graph():
    if "nc" in _CACHE:
        return _CACHE["nc"]

    import concourse.mybir as mybir
    import concourse.tile as tile
    from concourse import bacc
    from concourse.masks import make_identity

    dt = mybir.dt
    AF = mybir.ActivationFunctionType
    OP = mybir.AluOpType

    nc = bacc.Bacc("TRN2", target_bir_lowering=False, debug=False,
                   num_devices=NCORES)

    # ---------------- DRAM I/O (per-core shapes) ----------------
    # all big tensors pre-permuted on host so every DMA is a straight
    # [partition, contiguous-bytes] copy (no strided descriptors).
    xt2 = nc.dram_tensor("xt2", [2, 128, IC * S], dt.bfloat16,
                         kind="ExternalInput")
    adjt_d = nc.dram_tensor("adjt", [RSLOT, 128, 2 * S], dt.bfloat16,
                            kind="ExternalInput")
    w_d = nc.dram_tensor("w", [NL, RSLOT, 128, IC * FE], dt.bfloat16,
                         kind="ExternalInput")
    rec_d = nc.dram_tensor("rec", [S, 2 * RSLOT], dt.float32,
                           kind="ExternalInput")
    ebias_d = nc.dram_tensor("ebias", [S, 2 * NL], dt.float32,
                             kind="ExternalInput")
    mask_d = nc.dram_tensor("mask", [S, 2], dt.float32, kind="ExternalInput")
    wk_d = nc.dram_tensor("wk", [F, F], dt.bfloat16, kind="ExternalInput")
    wq_d = nc.dram_tensor("wq", [F, F], dt.bfloat16, kind="ExternalInput")
    wbil_d = nc.dram_tensor("wbil", [HD, HD], dt.bfloat16, kind="ExternalInput")
    wproj_d = nc.dram_tensor("wproj", [F, F], dt.bfloat16, kind="ExternalInput")
    bk_d = nc.dram_tensor("bk", [1, F], dt.bfloat16, kind="ExternalInput")
    bq_d = nc.dram_tensor("bq", [1, F], dt.bfloat16, kind="ExternalInput")
    bproj_d = nc.dram_tensor("bproj", [1, F], dt.bfloat16, kind="ExternalInput")
    qcol_d = nc.dram_tensor("qcol", [IC, S, 1], dt.bfloat16,
                            kind="ExternalInput")
    out_d = nc.dram_tensor("out", [1, F], dt.float32, kind="ExternalOutput")

    PAIRS = [[0, 1], [2, 3], [4, 5], [6, 7]]

    with tile.TileContext(nc) as tc:
        with (
            tc.tile_pool(name="const", bufs=1) as constp,
            tc.tile_pool(name="wpool", bufs=8) as wpool,
            tc.tile_pool(name="adjp", bufs=1) as adjp,
            tc.tile_pool(name="hidp", bufs=4) as hidp,
            tc.tile_pool(name="hT", bufs=1) as hTp,
            tc.tile_pool(name="payl", bufs=1) as paylp,
            tc.tile_pool(name="tail", bufs=4) as tailp,
            tc.tile_pool(name="misc", bufs=1) as miscp,
            tc.tile_pool(name="dram", bufs=1, space="DRAM") as dramp,
            tc.tile_pool(name="ps_hid", bufs=2, space="PSUM") as ps_hid,
            tc.tile_pool(name="ps_ld", bufs=2, space="PSUM") as ps_ld,
            tc.tile_pool(name="ps_intm", bufs=2, space="PSUM") as ps_intm,
        ):
            # layer-1 lhsT first in program order: the first combos need it
            cur_hT = {}
            for j in range(2):
                t = hTp.tile([128, IC * S], dt.bfloat16, name=f"hT{j}")
                nc.sync.dma_start(t[:], xt2[j])
                cur_hT[j] = t

            adj_tiles = {}
            w_cache = {}

            def load_w(l, r):
                if (l, r) not in w_cache:
                    t = wpool.tile([128, IC * FE], dt.bfloat16,
                                   name=f"w{l}_{r}", tag="wt")
                    nc.sync.dma_start(t[:], w_d[l, r])
                    w_cache[(l, r)] = t
                return w_cache[(l, r)]

            def get_adjT(r, j):
                if r not in adj_tiles:
                    t = adjp.tile([S, 2 * S], dt.bfloat16, name=f"adjT{r}")
                    nc.sync.dma_start(t[:], adjt_d[r])
                    adj_tiles[r] = t
                return adj_tiles[r][:, j * S:(j + 1) * S]

            # first combos' data ahead of all constant/warmup traffic
            load_w(0, 0)
            get_adjT(0, 0)
            load_w(0, 1)
            get_adjT(1, 0)

            ident_b = constp.tile([128, 128], dt.bfloat16, name="ident_b")
            make_identity(nc, ident_b)
            ident_f = constp.tile([128, 128], dt.float32, name="ident_f")
            make_identity(nc, ident_f)
            ones_row = constp.tile([1, 128], dt.bfloat16, name="ones_row")
            nc.vector.memset(ones_row, 1.0)
            one_sb = constp.tile([1, 1], dt.bfloat16, name="one_sb")
            nc.vector.memset(one_sb, 1.0)
            rec_sb = constp.tile([S, 2 * RSLOT], dt.float32, name="rec_sb")
            nc.sync.dma_start(rec_sb[:], rec_d[:])
            ebias_sb = constp.tile([S, 2 * NL], dt.float32, name="ebias_sb")
            nc.sync.dma_start(ebias_sb[:], ebias_d[:])
            mask_sb = constp.tile([S, 2], dt.float32, name="mask_sb")
            nc.sync.dma_start(mask_sb[:], mask_d[:])

            # collective bounce buffers (DRAM pool so Tile tracks deps)
            warm_in = dramp.tile([2, S, FE], dt.bfloat16, name="warm_in")
            warm_out = dramp.tile([2, S, FE], dt.bfloat16, name="warm_out")
            arin = [dramp.tile([2, S, FE], dt.bfloat16, name=f"arin{l}")
                    for l in range(NL)]
            arout = [dramp.tile([2, S, FE], dt.bfloat16, name=f"arout{l}")
                     for l in range(NL)]

            # warm up the CC rings with a full-size dummy so the first real
            # collective runs at steady-state latency (ENCD plan is per-size)
            warm_sb = constp.tile([S, FE], dt.bfloat16, name="warm_sb")
            nc.vector.memset(warm_sb, 0.0)
            for j in range(2):
                nc.sync.dma_start(warm_in[j][:], warm_sb[:])
            nc.gpsimd.collective_compute(
                "AllReduce", OP.add, replica_groups=PAIRS,
                ins=[warm_in.opt()], outs=[warm_out.opt()])

            payload = {}
            denacc = {}
            pend = [None]

            def rest(l, r, j, hid, adjT):
                ld = ps_ld.tile([S, 1], dt.float32, name=f"ld{l}_{r}_{j}",
                                tag="ld")
                nc.tensor.matmul(ld[:], lhsT=adjT, rhs=hid[:, F:FE],
                                 start=True, stop=True)
                col = 2 * r + j
                # tail: e = exp(logun*rec + bias); scr = e*rec
                tmul = tailp.tile([S, 1], dt.float32, name=f"tm{l}{r}{j}",
                                  tag="tm")
                nc.vector.tensor_mul(tmul[:], ld[:, 0:1],
                                     rec_sb[:, col:col + 1])
                bcol = 2 * l + (1 if r == RSLOT - 1 else 0)
                ee = tailp.tile([S, 1], dt.float32, name=f"ee{l}{r}{j}",
                                tag="ee")
                nc.scalar.activation(ee[:], tmul[:], AF.Exp,
                                     bias=ebias_sb[:, bcol:bcol + 1])
                scr = tailp.tile([S, 1], dt.float32, name=f"sc{l}{r}{j}",
                                 tag="sc")
                nc.vector.tensor_mul(scr[:], ee[:], rec_sb[:, col:col + 1])
                first = (l, j) not in payload
                if first:
                    payload[(l, j)] = paylp.tile([S, FE], dt.float32,
                                                 name=f"pay{l}_{j}")
                    denacc[(l, j)] = tailp.tile([S, 1], dt.float32,
                                                name=f"den{l}{j}", bufs=1)
                    nc.vector.tensor_copy(denacc[(l, j)][:], ee[:])
                else:
                    nc.vector.tensor_add(denacc[(l, j)][:], denacc[(l, j)][:],
                                         ee[:])
                pay = payload[(l, j)]
                for half in range(2):
                    c0 = half * 384
                    intm = ps_intm.tile([S, 384], dt.float32,
                                        name=f"in{l}{r}{j}{half}", tag="intm")
                    nc.tensor.matmul(intm[:], lhsT=adjT,
                                     rhs=hid[:, c0:c0 + 384],
                                     start=True, stop=True)
                    dst = pay[:, c0:c0 + 384]
                    if first:
                        nc.vector.tensor_scalar(dst, intm[:], scr[:], None,
                                                OP.mult)
                    else:
                        nc.vector.scalar_tensor_tensor(dst, intm[:], scr[:],
                                                       dst, OP.mult, OP.add)
                return (l, r, j)

            def combo(l, r, j):
                """Emit transform of (l,r,j); flush the PREVIOUS combo's
                aggregation behind it (software pipeline)."""
                wt = load_w(l, r)
                adjT = get_adjT(r, j)
                hid_ps = ps_hid.tile([S, FE], dt.float32,
                                     name=f"hps{l}_{r}_{j}", tag="hid")
                # consecutive matmuls share lhsT (one weight load per chunk)
                for ic in range(IC):
                    for c0, c1 in ((0, 512), (512, FE)):
                        nc.tensor.matmul(
                            hid_ps[:, c0:c1],
                            lhsT=cur_hT[j][:, ic * S:(ic + 1) * S],
                            rhs=wt[:, ic * FE + c0:ic * FE + c1],
                            start=(ic == 0), stop=(ic == IC - 1))
                hid = hidp.tile([S, FE], dt.bfloat16,
                                name=f"hid{l}_{r}_{j}", tag="hid")
                nc.scalar.copy(hid[:], hid_ps[:])
                prev = pend[0]
                pend[0] = (l, r, j, hid, adjT)
                if prev is not None:
                    return rest(*prev)
                return None

            def flush():
                prev = pend[0]
                pend[0] = None
                if prev is not None:
                    return rest(*prev)
                return None

            def ship_half(l, j):
                """Stage one batch's finished payload into the CC in-buffer."""
                pay = payload[(l, j)]
                nc.vector.tensor_copy(pay[:, F:FE], denacc[(l, j)][:])
                pyc = miscp.tile([S, FE], dt.bfloat16, name=f"pyc{l}{j}",
                                 tag="pyc", bufs=2)
                nc.scalar.copy(pyc[:], pay[:])
                nc.sync.dma_start(arin[l][j][:], pyc[:])

            def ship(l):
                """One merged pairwise AllReduce: [full-A | full-B]."""
                ship_half(l, 1)
                nc.gpsimd.collective_compute(
                    "AllReduce", OP.add, replica_groups=PAIRS,
                    ins=[arin[l].opt()], outs=[arout[l].opt()])

            def h2_prep(l, j):
                raw = miscp.tile([S, FE], dt.bfloat16, name=f"raw{l}{j}",
                                 tag="raw", bufs=2)
                nc.sync.dma_start(raw[:], arout[l][j][:])
                rd = miscp.tile([S, 1], dt.float32, name=f"rd{l}{j}", tag="rd")
                nc.vector.reciprocal(rd[:], raw[:, F:FE])
                h2 = miscp.tile([S, F], dt.bfloat16, name=f"h2_{l}{j}",
                                tag="h2")
                t = hTp.tile([128, IC * S], dt.bfloat16, name=f"h2T{l}{j}")
                # chunked relu->transpose so the next layer's first matmul
                # can start as soon as chunk 0 lands
                for ic in range(IC):
                    sl = slice(ic * 128, (ic + 1) * 128)
                    nc.scalar.activation(h2[:, sl], raw[:, sl], AF.Relu,
                                         scale=rd[:])
                    tp = ps_ld.tile([128, 128], dt.bfloat16,
                                    name=f"tp{l}{j}_{ic}", tag="ld")
                    nc.tensor.transpose(tp[:], h2[:, sl], ident_b[:])
                    nc.scalar.copy(t[:, ic * S:(ic + 1) * S], tp[:])
                cur_hT[j] = t

            qst = {}

            def q_loads():
                # attention q-side inputs: small, prefetch mid-stream
                qc = []
                for ic in range(IC):
                    t = constp.tile([S, 1], dt.bfloat16, name=f"qc{ic}")
                    nc.sync.dma_start(t[:], qcol_d[ic])
                    qc.append(t)
                bq_sb = constp.tile([1, F], dt.bfloat16, name="bq_sb")
                nc.sync.dma_start(bq_sb[:], bq_d[:])
                wbil_sb = constp.tile([HD, HD], dt.bfloat16, name="wbil_sb")
                nc.sync.dma_start(wbil_sb[:], wbil_d[:])
                wqts = []
                for ic in range(IC):
                    wqt = wpool.tile([128, F], dt.bfloat16, name=f"wq{ic}",
                                     tag="wqt", bufs=IC)
                    nc.sync.dma_start(wqt[:], wq_d[ic * 128:(ic + 1) * 128, :])
                    wqts.append(wqt)
                qst.update(qc=qc, bq=bq_sb, wbil=wbil_sb, wq=wqts)

            def q_compute():
                qc, bq_sb, wbil_sb, wqts = (qst["qc"], qst["bq"], qst["wbil"],
                                            qst["wq"])
                one_f = constp.tile([1, 1], dt.bfloat16, name="one_f")
                nc.vector.memset(one_f, 1.0)
                qxT_ps = ps_intm.tile([HD, NH, 4], dt.float32, name="qxT_ps",
                                      tag="intm")
                for hh in range(NH):
                    for ic in range(IC):
                        nc.tensor.matmul(
                            qxT_ps[:, hh, 0:1],
                            lhsT=wqts[ic][:, hh * HD:(hh + 1) * HD],
                            rhs=qc[ic][:],
                            start=(ic == 0), stop=False)
                    nc.tensor.matmul(qxT_ps[:, hh, 0:1],
                                     lhsT=bq_sb[:, hh * HD:(hh + 1) * HD],
                                     rhs=one_f[:], start=False, stop=True)
                qxT = constp.tile([HD, NH], dt.bfloat16, name="qxT")
                nc.scalar.copy(qxT[:], qxT_ps[:, :, 0])
                qw_ps = ps_intm.tile([HD, NH, 4], dt.float32, name="qw_ps",
                                     tag="intm")
                for hh in range(NH):
                    nc.tensor.matmul(qw_ps[:, hh, 0:1], lhsT=wbil_sb[:],
                                     rhs=qxT[:, hh:hh + 1], start=True,
                                     stop=True)
                qwT = constp.tile([HD, NH], dt.bfloat16, name="qwT")
                nc.scalar.copy(qwT[:], qw_ps[:, :, 0])
                qst["qwT"] = qwT

            # attention weight tiles (loaded during layer 2)
            att = {}

            def _load_att_weights():
                bk_sb = constp.tile([1, F], dt.bfloat16, name="bk_sb")
                nc.sync.dma_start(bk_sb[:], bk_d[:])
                bp_sb = constp.tile([1, F], dt.bfloat16, name="bp_sb")
                nc.sync.dma_start(bp_sb[:], bproj_d[:])
                wkts, wpts = [], []
                for ic in range(IC):
                    wkt = wpool.tile([128, F], dt.bfloat16, name=f"wk{ic}",
                                     tag="wkt", bufs=IC)
                    nc.sync.dma_start(wkt[:], wk_d[ic * 128:(ic + 1) * 128, :])
                    wkts.append(wkt)
                    wpt = wpool.tile([128, F], dt.bfloat16, name=f"wp{ic}",
                                     tag="wpt", bufs=IC)
                    nc.sync.dma_start(wpt[:],
                                      wproj_d[ic * 128:(ic + 1) * 128, :])
                    wpts.append(wpt)
                att["bk"] = bk_sb
                att["bp"] = bp_sb
                att["wk"] = wkts
                att["wp"] = wpts

            def layer(l, hooks):
                seqA = [(r, 0) for r in range(RSLOT)]
                seqB = [(r, 1) for r in range(RSLOT)]
                seq = []
                ia = ib = 0
                while ia < len(seqA) or ib < len(seqB):
                    if ia < len(seqA) and (ia - ib < LEAD or ib >= len(seqB)):
                        seq.append(seqA[ia])
                        ia += 1
                    else:
                        seq.append(seqB[ib])
                        ib += 1
                last_a = (l, RSLOT - 1, 0)
                for k, (r, j) in enumerate(seq):
                    if k in hooks:
                        hooks[k]()
                    if combo(l, r, j) == last_a:
                        ship_half(l, 0)   # A done early: overlap its staging
                done = flush()
                if done == last_a:
                    ship_half(l, 0)
                ship(l)

            layer(0, {3: q_loads, 14: q_compute})
            h2_prep(0, 0)
            layer(1, {1: lambda: h2_prep(0, 1), 6: _load_att_weights})

            # =================== attention (own batch via mask) ==========
            qwT = qst["qwT"]
            r0 = miscp.tile([S, FE], dt.bfloat16, name="r0", tag="raw", bufs=2)
            nc.sync.dma_start(r0[:], arout[1][0][:])
            r1 = miscp.tile([S, FE], dt.bfloat16, name="r1", tag="raw", bufs=2)
            nc.sync.dma_start(r1[:], arout[1][1][:])
            rawf = miscp.tile([S, FE], dt.float32, name="rawf", tag="rawf")
            nc.vector.tensor_scalar(rawf[:], r0[:], mask_sb[:, 0:1], None,
                                    OP.mult)
            nc.vector.scalar_tensor_tensor(rawf[:], r1[:], mask_sb[:, 1:2],
                                           rawf[:], OP.mult, OP.add)
            rd = miscp.tile([S, 1], dt.float32, name="rdf", tag="rd")
            nc.vector.reciprocal(rd[:], rawf[:, F:FE])
            hf = miscp.tile([S, F], dt.bfloat16, name="hf", tag="h2")
            nc.scalar.activation(hf[:], rawf[:, :F], AF.Relu, scale=rd[:])
            hfT = hTp.tile([128, IC * S], dt.bfloat16, name="hfT")
            for ic in range(IC):
                tp = ps_ld.tile([128, 128], dt.bfloat16, name=f"ftp{ic}",
                                tag="ld")
                nc.tensor.transpose(tp[:], hf[:, ic * 128:(ic + 1) * 128],
                                    ident_b[:])
                nc.scalar.copy(hfT[:, ic * S:(ic + 1) * S], tp[:])

            bk_sb, bp_sb = att["bk"], att["bp"]
            wkts, wpts = att["wk"], att["wp"]
            # kx = hf @ wk + bk   [S, 768]
            kx_ps = ps_hid.tile([S, F], dt.float32, name="kx_ps", tag="hid")
            for ic in range(IC):
                lhsT = hfT[:, ic * S:(ic + 1) * S]
                nc.tensor.matmul(kx_ps[:, 0:512], lhsT=lhsT,
                                 rhs=wkts[ic][:, 0:512],
                                 start=(ic == 0), stop=False)
                nc.tensor.matmul(kx_ps[:, 512:F], lhsT=lhsT,
                                 rhs=wkts[ic][:, 512:F],
                                 start=(ic == 0), stop=False)
            nc.tensor.matmul(kx_ps[:, 0:512], lhsT=ones_row[:],
                             rhs=bk_sb[:, 0:512], start=False, stop=True)
            nc.tensor.matmul(kx_ps[:, 512:F], lhsT=ones_row[:],
                             rhs=bk_sb[:, 512:F], start=False, stop=True)
            kx = miscp.tile([S, F], dt.bfloat16, name="kx", tag="kx")
            nc.scalar.copy(kx[:], kx_ps[:])
            # kxT per head directly: kxT_h = sum_ic wk[ic,h]^T @ hfT[ic]
            kxT = miscp.tile([HD, NH * S], dt.bfloat16, name="kxT", tag="kxT")
            for hh in range(NH):
                ktp = ps_intm.tile([HD, S], dt.float32, name=f"ktp{hh}",
                                   tag="intm")
                for ic in range(IC):
                    nc.tensor.matmul(
                        ktp[:],
                        lhsT=wkts[ic][:, hh * HD:(hh + 1) * HD],
                        rhs=hfT[:, ic * S:(ic + 1) * S],
                        start=(ic == 0), stop=False)
                nc.tensor.matmul(ktp[:],
                                 lhsT=bk_sb[:, hh * HD:(hh + 1) * HD],
                                 rhs=ones_row[:], start=False, stop=True)
                nc.scalar.copy(kxT[:, hh * S:(hh + 1) * S], ktp[:])

            # scoreT[:,h] = kx_h @ qwT_h    [128, 8]
            sc_ps = ps_intm.tile([S, NH, 4], dt.float32, name="sc_ps",
                                 tag="intm")
            for hh in range(NH):
                nc.tensor.matmul(sc_ps[:, hh, 0:1],
                                 lhsT=kxT[:, hh * S:(hh + 1) * S],
                                 rhs=qwT[:, hh:hh + 1], start=True,
                                 stop=True)
            sc_sb = miscp.tile([S, NH], dt.float32, name="sc_sb", tag="scb", bufs=3)
            nc.scalar.copy(sc_sb[:], sc_ps[:, :, 0])
            # score rows [8, 128]
            srow_ps = ps_ld.tile([NH, S], dt.float32, name="srow", tag="ld")
            nc.tensor.transpose(srow_ps[:], sc_sb[:], ident_f[:])
            negmax = miscp.tile([NH, 1], dt.float32, name="negmax", tag="sm", bufs=3)
            nc.vector.tensor_reduce(negmax[:], srow_ps[:],
                                    mybir.AxisListType.X, OP.max,
                                    negate=True)
            esc = miscp.tile([NH, S], dt.float32, name="esc", tag="esc", bufs=2)
            sumexp = miscp.tile([NH, 1], dt.float32, name="sumexp", tag="sm", bufs=3)
            nc.scalar.activation(esc[:], srow_ps[:], AF.Exp, bias=negmax[:],
                                 accum_out=sumexp[:])
            rsm = miscp.tile([NH, 1], dt.float32, name="rsm", tag="sm", bufs=3)
            nc.vector.reciprocal(rsm[:], sumexp[:])
            attn = miscp.tile([NH, S], dt.bfloat16, name="attn", tag="esc", bufs=2)
            nc.vector.tensor_scalar_mul(attn[:], esc[:], rsm[:])
            # attnT [128, 8]
            at_ps = ps_ld.tile([S, NH], dt.bfloat16, name="at_ps", tag="ld")
            nc.tensor.transpose(at_ps[:], attn[:], ident_b[:NH, :NH])
            attnT = miscp.tile([S, NH], dt.bfloat16, name="attnT", tag="scb", bufs=3)
            nc.scalar.copy(attnT[:], at_ps[:])
            # o[0, h*96:(h+1)*96] = attn_h @ kx_h  (bank-safe [1,8,128] tiles)
            o_psA = ps_intm.tile([1, 4, 128], dt.float32, name="o_psA",
                                 tag="intm")
            o_psB = ps_intm.tile([1, 4, 128], dt.float32, name="o_psB",
                                 tag="intm")
            for hh in range(NH):
                tgt = o_psA if hh < 4 else o_psB
                nc.tensor.matmul(tgt[:, hh % 4, :HD],
                                 lhsT=attnT[:, hh:hh + 1],
                                 rhs=kx[:, hh * HD:(hh + 1) * HD],
                                 start=True, stop=True)
            o_sb = miscp.tile([1, F], dt.bfloat16, name="o_sb", tag="qx")
            nc.scalar.copy(o_sb[:, 0:384], o_psA[:, :, :HD])
            nc.scalar.copy(o_sb[:, 384:F], o_psB[:, :, :HD])
            # oT [128, 6]
            oT_ps = ps_ld.tile([S, IC, 4], dt.bfloat16, name="oT_ps", tag="ld")
            for ic in range(IC):
                nc.tensor.transpose(oT_ps[:, ic, 0:1],
                                    o_sb[:, ic * 128:(ic + 1) * 128],
                                    ident_b[:1, :1])
            oT = miscp.tile([S, IC], dt.bfloat16, name="oT", tag="scb", bufs=3)
            nc.scalar.copy(oT[:], oT_ps[:, :, 0])
            # res = o @ wproj + bproj
            res_ps = ps_hid.tile([1, 512], dt.float32, name="res_ps",
                                 tag="hid")
            res_ps2 = ps_hid.tile([1, 256], dt.float32, name="res_ps2",
                                  tag="hid")
            for ic in range(IC):
                nc.tensor.matmul(res_ps[:], lhsT=oT[:, ic:ic + 1],
                                 rhs=wpts[ic][:, 0:512],
                                 start=(ic == 0), stop=False)
                nc.tensor.matmul(res_ps2[:], lhsT=oT[:, ic:ic + 1],
                                 rhs=wpts[ic][:, 512:F],
                                 start=(ic == 0), stop=False)
            nc.tensor.matmul(res_ps[:], lhsT=one_sb[:], rhs=bp_sb[:, 0:512],
                             start=False, stop=True)
            nc.tensor.matmul(res_ps2[:], lhsT=one_sb[:], rhs=bp_sb[:, 512:F],
                             start=False, stop=True)
            res_sb = miscp.tile([1, F], dt.float32, name="res_sb", tag="res")
            nc.scalar.copy(res_sb[:, 0:512], res_ps[:])
            nc.scalar.copy(res_sb[:, 512:F], res_ps2[:])
            nc.sync.dma_start(out_d[:], res_sb[:])

    nc.compile()
    _CACHE["nc"] = nc
    return nc


def _prep_inputs(x, adj, q, w_rgcn, score_w, score_b, wk, bk, wq, bq, wbil,
                 wproj, bproj):
    f32 = np.float32
    x = np.asarray(x, f32)
    adj = np.asarray(adj, f32)
    q = np.asarray(q, f32)
    w_rgcn = np.asarray(w_rgcn, f32)
    score_w = np.asarray(score_w, f32)
    score_b = np.asarray(score_b, f32)

    u = np.einsum("lrio,lo->lri", w_rgcn, score_w).astype(f32)
    w_ext = np.concatenate([w_rgcn, u[..., None]], axis=-1)  # [2,41,768,769]
    # pre-permute: [l, r, i, f] -> [l, r, p, (c f)] with i = c*128 + p
    w_perm = np.ascontiguousarray(
        w_ext.reshape(NL, R, IC, 128, FE).transpose(0, 1, 3, 2, 4)
        .reshape(NL, R, 128, IC * FE)).astype(bf16)

    # adj rowsum reciprocal (device never computes denominators)
    den = adj.sum(axis=3)                                    # [B, R, S]
    rec = (1.0 / np.where(den == 0.0, 1.0, den)).astype(f32)

    shared = {
        "wk": np.asarray(wk, f32).astype(bf16),
        "wq": np.asarray(wq, f32).astype(bf16),
        "wbil": np.asarray(wbil, f32).astype(bf16),
        "wproj": np.asarray(wproj, f32).astype(bf16),
        "bk": np.asarray(bk, f32).reshape(1, F).astype(bf16),
        "bq": np.asarray(bq, f32).reshape(1, F).astype(bf16),
        "bproj": np.asarray(bproj, f32).reshape(1, F).astype(bf16),
    }

    in_maps = []
    for c in range(NCORES):
        p, h = c // 2, c % 2
        A, Bb = 2 * p, 2 * p + 1
        rsel = list(range(0, 20) if h == 0 else range(20, 40)) + [R - 1]
        w_c = w_perm[:, rsel]                                # [2,21,128,IC*FE]
        adjt_c = np.empty((RSLOT, 128, 2 * S), f32)
        rec_c = np.empty((S, 2 * RSLOT), f32)
        for j, bb in enumerate((A, Bb)):
            adjt_c[:, :, j * S:(j + 1) * S] = adj[bb, rsel].transpose(0, 2, 1)
            rec_c[:, 2 * np.arange(RSLOT) + j] = rec[bb, rsel].T
        eb_c = np.empty((S, 2 * NL), f32)
        for l in range(NL):
            eb_c[:, 2 * l] = score_b[l]
            eb_c[:, 2 * l + 1] = score_b[l] + np.log(0.5)
        mask_c = np.zeros((S, 2), f32)
        mask_c[:, h] = 1.0
        xh = np.empty((2, 128, IC * S), f32)
        for j, bb in enumerate((A, Bb)):
            xh[j] = (x[bb].T.reshape(IC, 128, S).transpose(1, 0, 2)
                     .reshape(128, IC * S))
        m = dict(shared)
        m["w"] = np.ascontiguousarray(w_c)
        m["adjt"] = np.ascontiguousarray(adjt_c).astype(bf16)
        m["rec"] = rec_c
        m["ebias"] = eb_c
        m["mask"] = mask_c
        m["qcol"] = q[c].reshape(IC, S, 1).astype(bf16)
        m["xt2"] = xh.astype(bf16)
        in_maps.append(m)
    return in_maps


def kernel(**inputs) -> np.ndarray:
    from concourse.bass_utils import run_bass_kernel_spmd

    nc = _build_graph()
    in_maps = _prep_inputs(**inputs)
    res = run_bass_kernel_spmd(nc, in_maps, core_ids=list(range(NCORES)))
    outs = [np.asarray(res.results[c]["out"], np.float32) for c in range(NCORES)]
    return np.stack(outs)  # [8, 1, 768]


# revision 17
# speedup vs baseline: 1.0599x; 1.0599x over previous
"""AERGCN (2-layer R-GCN + bilinear attention pool) on 8 TRN2 NeuronCores.

Sharding: pair-hybrid. Cores are paired (2p, 2p+1); pair p owns batches
A=2p, B=2p+1. Within a pair the 41 relations split 20/20 (even core: rels
0-19, odd: 20-39) and relation 40 is computed by BOTH cores at half weight
(the 0.5 is folded into the relation-softmax exp bias as +ln 0.5), keeping
the graph fully SPMD-symmetric. Each layer runs one stream of 42 (rel,
batch) combos strictly alternating A/B (A leads by 2) so each weight tile
is DMA'd once and consumed by both batches back-to-back, keeping weight
demand at a steady 1 tile / 2 combos. Per layer, ONE pairwise AllReduce
([2,S,769] bf16: full-A | full-B payloads) reconstitutes the relation
softmax for both batches in a single collective (fixed CC cost ~11us paid
once per layer, not twice). 1/denom is precomputed on host, so combos
issue no GpSimd work and CC triggers can't head-of-line-block the pipe.
After layer 2 each core runs the attention pool for its own batch
(selected from the AllReduce output by a data-driven mask to stay SPMD).

Matmuls in bf16 (f32 PSUM). Per-combo pipeline:
  hidden = h @ [W_r | W_r @ score_w]          (12 accumulating matmuls)
  logun = adj @ u                             (1 matmul, N=1, lhsT=adjT)
  e = exp(logun*rec + bias); scr = e*rec      (rec = 1/denom from host)
  payload += scr * (adj @ hidden)             (2 matmuls N=384; DVE)
"""

import os
import sys

# The Bass NEFF executes through the axon PJRT backend; if the caller pinned
# jax to cpu before we ever import jax, lift the pin so axon devices resolve.
if "jax" not in sys.modules and os.environ.get("JAX_PLATFORMS") == "cpu":
    os.environ["JAX_PLATFORMS"] = ""

import numpy as np
import ml_dtypes

bf16 = ml_dtypes.bfloat16

B, S, F, R, NL = 8, 128, 768, 41, 2
NH, HD, EMB = 8, 96, 768
NCORES, IC = 8, 6
FE = F + 1      # 769: W with appended u column
RSLOT = 21      # 20 private relations + shared relation 40 (half weight)
LEAD = 2        # batch A runs this many relations ahead of batch B

_CACHE = {}


def _build_graph():
    if "nc" in _CACHE:
        return _CACHE["nc"]

    import concourse.mybir as mybir
    import concourse.tile as tile
    from concourse import bacc
    from concourse.masks import make_identity

    dt = mybir.dt
    AF = mybir.ActivationFunctionType
    OP = mybir.AluOpType

    nc = bacc.Bacc("TRN2", target_bir_lowering=False, debug=False,
                   num_devices=NCORES)

    # ---------------- DRAM I/O (per-core shapes) ----------------
    # all big tensors pre-permuted on host so every DMA is a straight
    # [partition, contiguous-bytes] copy (no strided descriptors).
    xt2 = nc.dram_tensor("xt2", [2, 128, IC * S], dt.bfloat16,
                         kind="ExternalInput")
    adjt_d = nc.dram_tensor("adjt", [RSLOT, 128, 2 * S], dt.bfloat16,
                            kind="ExternalInput")
    w_d = nc.dram_tensor("w", [NL, RSLOT, 128, IC * FE], dt.bfloat16,
                         kind="ExternalInput")
    rec_d = nc.dram_tensor("rec", [S, 2 * RSLOT], dt.float32,
                           kind="ExternalInput")
    ebias_d = nc.dram_tensor("ebias", [S, 2 * NL], dt.float32,
                             kind="ExternalInput")
    mask_d = nc.dram_tensor("mask", [S, 2], dt.float32, kind="ExternalInput")
    wk_d = nc.dram_tensor("wk", [F, F], dt.bfloat16, kind="ExternalInput")
    wq_d = nc.dram_tensor("wq", [F, F], dt.bfloat16, kind="ExternalInput")
    wbil_d = nc.dram_tensor("wbil", [HD, HD], dt.bfloat16, kind="ExternalInput")
    wproj_d = nc.dram_tensor("wproj", [F, F], dt.bfloat16, kind="ExternalInput")
    bk_d = nc.dram_tensor("bk", [1, F], dt.bfloat16, kind="ExternalInput")
    bq_d = nc.dram_tensor("bq", [1, F], dt.bfloat16, kind="ExternalInput")
    bproj_d = nc.dram_tensor("bproj", [1, F], dt.bfloat16, kind="ExternalInput")
    qcol_d = nc.dram_tensor("qcol", [IC, S, 1], dt.bfloat16,
                            kind="ExternalInput")
    out_d = nc.dram_tensor("out", [1, F], dt.float32, kind="ExternalOutput")

    PAIRS = [[0, 1], [2, 3], [4, 5], [6, 7]]

    with tile.TileContext(nc) as tc:
        with (
            tc.tile_pool(name="const", bufs=1) as constp,
            tc.tile_pool(name="wpool", bufs=5) as wpool,
            tc.tile_pool(name="adjp", bufs=1) as adjp,
            tc.tile_pool(name="hidp", bufs=4) as hidp,
            tc.tile_pool(name="hT", bufs=1) as hTp,
            tc.tile_pool(name="payl", bufs=1) as paylp,
            tc.tile_pool(name="tail", bufs=4) as tailp,
            tc.tile_pool(name="misc", bufs=1) as miscp,
            tc.tile_pool(name="dram", bufs=1, space="DRAM") as dramp,
            tc.tile_pool(name="ps_hid", bufs=2, space="PSUM") as ps_hid,
            tc.tile_pool(name="ps_ld", bufs=2, space="PSUM") as ps_ld,
            tc.tile_pool(name="ps_intm", bufs=2, space="PSUM") as ps_intm,
        ):
            # layer-1 lhsT first in program order: the first combos need it
            cur_hT = {}
            for j in range(2):
                t = hTp.tile([128, IC * S], dt.bfloat16, name=f"hT{j}")
                nc.sync.dma_start(t[:], xt2[j])
                cur_hT[j] = t

            adj_tiles = {}
            w_cache = {}

            def load_w(l, r):
                if (l, r) not in w_cache:
                    t = wpool.tile([128, IC * FE], dt.bfloat16,
                                   name=f"w{l}_{r}", tag="wt")
                    nc.sync.dma_start(t[:], w_d[l, r])
                    w_cache[(l, r)] = t
                return w_cache[(l, r)]

            def get_adjT(r, j):
                if r not in adj_tiles:
                    t = adjp.tile([S, 2 * S], dt.bfloat16, name=f"adjT{r}")
                    nc.sync.dma_start(t[:], adjt_d[r])
                    adj_tiles[r] = t
                return adj_tiles[r][:, j * S:(j + 1) * S]

            # first combos' data ahead of all constant/warmup traffic
            load_w(0, 0)
            get_adjT(0, 0)
            load_w(0, 1)
            get_adjT(1, 0)

            ident_b = constp.tile([128, 128], dt.bfloat16, name="ident_b")
            make_identity(nc, ident_b)
            ident_f = constp.tile([128, 128], dt.float32, name="ident_f")
            make_identity(nc, ident_f)
            ones_row = constp.tile([1, 128], dt.bfloat16, name="ones_row")
            nc.vector.memset(ones_row, 1.0)
            one_sb = constp.tile([1, 1], dt.bfloat16, name="one_sb")
            nc.vector.memset(one_sb, 1.0)
            rec_sb = constp.tile([S, 2 * RSLOT], dt.float32, name="rec_sb")
            nc.sync.dma_start(rec_sb[:], rec_d[:])
            ebias_sb = constp.tile([S, 2 * NL], dt.float32, name="ebias_sb")
            nc.sync.dma_start(ebias_sb[:], ebias_d[:])
            mask_sb = constp.tile([S, 2], dt.float32, name="mask_sb")
            nc.sync.dma_start(mask_sb[:], mask_d[:])

            # collective bounce buffers (DRAM pool so Tile tracks deps)
            warm_in = dramp.tile([8, 16], dt.bfloat16, name="warm_in")
            warm_out = dramp.tile([8, 16], dt.bfloat16, name="warm_out")
            arin = [dramp.tile([2, S, FE], dt.bfloat16, name=f"arin{l}")
                    for l in range(NL)]
            arout = [dramp.tile([2, S, FE], dt.bfloat16, name=f"arout{l}")
                     for l in range(NL)]

            # warm up the CC rings before the first real collective
            warm_sb = constp.tile([8, 16], dt.bfloat16, name="warm_sb")
            nc.vector.memset(warm_sb, 1.0)
            nc.sync.dma_start(warm_in[:], warm_sb[:])
            nc.gpsimd.collective_compute(
                "AllReduce", OP.add, replica_groups=PAIRS,
                ins=[warm_in.opt()], outs=[warm_out.opt()])

            payload = {}
            denacc = {}
            pend = [None]

            def rest(l, r, j, hid, adjT):
                ld = ps_ld.tile([S, 1], dt.float32, name=f"ld{l}_{r}_{j}",
                                tag="ld")
                nc.tensor.matmul(ld[:], lhsT=adjT, rhs=hid[:, F:FE],
                                 start=True, stop=True)
                col = 2 * r + j
                # tail: e = exp(logun*rec + bias); scr = e*rec
                tmul = tailp.tile([S, 1], dt.float32, name=f"tm{l}{r}{j}",
                                  tag="tm")
                nc.vector.tensor_mul(tmul[:], ld[:, 0:1],
                                     rec_sb[:, col:col + 1])
                bcol = 2 * l + (1 if r == RSLOT - 1 else 0)
                ee = tailp.tile([S, 1], dt.float32, name=f"ee{l}{r}{j}",
                                tag="ee")
                nc.scalar.activation(ee[:], tmul[:], AF.Exp,
                                     bias=ebias_sb[:, bcol:bcol + 1])
                scr = tailp.tile([S, 1], dt.float32, name=f"sc{l}{r}{j}",
                                 tag="sc")
                nc.vector.tensor_mul(scr[:], ee[:], rec_sb[:, col:col + 1])
                first = (l, j) not in payload
                if first:
                    payload[(l, j)] = paylp.tile([S, FE], dt.float32,
                                                 name=f"pay{l}_{j}")
                    denacc[(l, j)] = tailp.tile([S, 1], dt.float32,
                                                name=f"den{l}{j}", bufs=1)
                    nc.vector.tensor_copy(denacc[(l, j)][:], ee[:])
                else:
                    nc.vector.tensor_add(denacc[(l, j)][:], denacc[(l, j)][:],
                                         ee[:])
                pay = payload[(l, j)]
                for half in range(2):
                    c0 = half * 384
                    intm = ps_intm.tile([S, 384], dt.float32,
                                        name=f"in{l}{r}{j}{half}", tag="intm")
                    nc.tensor.matmul(intm[:], lhsT=adjT,
                                     rhs=hid[:, c0:c0 + 384],
                                     start=True, stop=True)
                    dst = pay[:, c0:c0 + 384]
                    if first:
                        nc.vector.tensor_scalar(dst, intm[:], scr[:], None,
                                                OP.mult)
                    else:
                        nc.vector.scalar_tensor_tensor(dst, intm[:], scr[:],
                                                       dst, OP.mult, OP.add)
                return (l, r, j)

            def combo(l, r, j):
                """Emit transform of (l,r,j); flush the PREVIOUS combo's
                aggregation behind it (software pipeline)."""
                wt = load_w(l, r)
                adjT = get_adjT(r, j)
                hid_ps = ps_hid.tile([S, FE], dt.float32,
                                     name=f"hps{l}_{r}_{j}", tag="hid")
                for c0, c1 in ((0, 512), (512, FE)):
                    for ic in range(IC):
                        nc.tensor.matmul(
                            hid_ps[:, c0:c1],
                            lhsT=cur_hT[j][:, ic * S:(ic + 1) * S],
                            rhs=wt[:, ic * FE + c0:ic * FE + c1],
                            start=(ic == 0), stop=(ic == IC - 1))
                hid = hidp.tile([S, FE], dt.bfloat16,
                                name=f"hid{l}_{r}_{j}", tag="hid")
                nc.scalar.copy(hid[:], hid_ps[:])
                prev = pend[0]
                pend[0] = (l, r, j, hid, adjT)
                if prev is not None:
                    return rest(*prev)
                return None

            def flush():
                prev = pend[0]
                pend[0] = None
                if prev is not None:
                    return rest(*prev)
                return None

            def ship_half(l, j):
                """Stage one batch's finished payload into the CC in-buffer."""
                pay = payload[(l, j)]
                nc.vector.tensor_copy(pay[:, F:FE], denacc[(l, j)][:])
                pyc = miscp.tile([S, FE], dt.bfloat16, name=f"pyc{l}{j}",
                                 tag="pyc", bufs=2)
                nc.scalar.copy(pyc[:], pay[:])
                nc.sync.dma_start(arin[l][j][:], pyc[:])

            def ship(l):
                """One merged pairwise AllReduce: [full-A | full-B]."""
                ship_half(l, 1)
                nc.gpsimd.collective_compute(
                    "AllReduce", OP.add, replica_groups=PAIRS,
                    ins=[arin[l].opt()], outs=[arout[l].opt()])

            def h2_prep(l, j):
                raw = miscp.tile([S, FE], dt.bfloat16, name=f"raw{l}{j}",
                                 tag="raw", bufs=2)
                nc.sync.dma_start(raw[:], arout[l][j][:])
                rd = miscp.tile([S, 1], dt.float32, name=f"rd{l}{j}", tag="rd")
                nc.vector.reciprocal(rd[:], raw[:, F:FE])
                h2 = miscp.tile([S, F], dt.bfloat16, name=f"h2_{l}{j}",
                                tag="h2")
                t = hTp.tile([128, IC * S], dt.bfloat16, name=f"h2T{l}{j}")
                # chunked relu->transpose so the next layer's first matmul
                # can start as soon as chunk 0 lands
                for ic in range(IC):
                    sl = slice(ic * 128, (ic + 1) * 128)
                    nc.scalar.activation(h2[:, sl], raw[:, sl], AF.Relu,
                                         scale=rd[:])
                    tp = ps_ld.tile([128, 128], dt.bfloat16,
                                    name=f"tp{l}{j}_{ic}", tag="ld")
                    nc.tensor.transpose(tp[:], h2[:, sl], ident_b[:])
                    nc.scalar.copy(t[:, ic * S:(ic + 1) * S], tp[:])
                cur_hT[j] = t

            qst = {}

            def q_loads():
                # attention q-side inputs: small, prefetch mid-stream
                qc = []
                for ic in range(IC):
                    t = constp.tile([S, 1], dt.bfloat16, name=f"qc{ic}")
                    nc.sync.dma_start(t[:], qcol_d[ic])
                    qc.append(t)
                bq_sb = constp.tile([1, F], dt.bfloat16, name="bq_sb")
                nc.sync.dma_start(bq_sb[:], bq_d[:])
                wbil_sb = constp.tile([HD, HD], dt.bfloat16, name="wbil_sb")
                nc.sync.dma_start(wbil_sb[:], wbil_d[:])
                wqts = []
                for ic in range(IC):
                    wqt = wpool.tile([128, F], dt.bfloat16, name=f"wq{ic}",
                                     tag="wqt", bufs=IC)
                    nc.sync.dma_start(wqt[:], wq_d[ic * 128:(ic + 1) * 128, :])
                    wqts.append(wqt)
                qst.update(qc=qc, bq=bq_sb, wbil=wbil_sb, wq=wqts)

            def q_compute():
                qc, bq_sb, wbil_sb, wqts = (qst["qc"], qst["bq"], qst["wbil"],
                                            qst["wq"])
                one_f = constp.tile([1, 1], dt.bfloat16, name="one_f")
                nc.vector.memset(one_f, 1.0)
                qxT_ps = ps_intm.tile([HD, NH, 4], dt.float32, name="qxT_ps",
                                      tag="intm")
                for hh in range(NH):
                    for ic in range(IC):
                        nc.tensor.matmul(
                            qxT_ps[:, hh, 0:1],
                            lhsT=wqts[ic][:, hh * HD:(hh + 1) * HD],
                            rhs=qc[ic][:],
                            start=(ic == 0), stop=False)
                    nc.tensor.matmul(qxT_ps[:, hh, 0:1],
                                     lhsT=bq_sb[:, hh * HD:(hh + 1) * HD],
                                     rhs=one_f[:], start=False, stop=True)
                qxT = constp.tile([HD, NH], dt.bfloat16, name="qxT")
                nc.scalar.copy(qxT[:], qxT_ps[:, :, 0])
                qw_ps = ps_intm.tile([HD, NH, 4], dt.float32, name="qw_ps",
                                     tag="intm")
                for hh in range(NH):
                    nc.tensor.matmul(qw_ps[:, hh, 0:1], lhsT=wbil_sb[:],
                                     rhs=qxT[:, hh:hh + 1], start=True,
                                     stop=True)
                qwT = constp.tile([HD, NH], dt.bfloat16, name="qwT")
                nc.scalar.copy(qwT[:], qw_ps[:, :, 0])
                qst["qwT"] = qwT

            # attention weight tiles (loaded during layer 2)
            att = {}

            def _load_att_weights():
                bk_sb = constp.tile([1, F], dt.bfloat16, name="bk_sb")
                nc.sync.dma_start(bk_sb[:], bk_d[:])
                bp_sb = constp.tile([1, F], dt.bfloat16, name="bp_sb")
                nc.sync.dma_start(bp_sb[:], bproj_d[:])
                wkts, wpts = [], []
                for ic in range(IC):
                    wkt = wpool.tile([128, F], dt.bfloat16, name=f"wk{ic}",
                                     tag="wkt", bufs=IC)
                    nc.sync.dma_start(wkt[:], wk_d[ic * 128:(ic + 1) * 128, :])
                    wkts.append(wkt)
                    wpt = wpool.tile([128, F], dt.bfloat16, name=f"wp{ic}",
                                     tag="wpt", bufs=IC)
                    nc.sync.dma_start(wpt[:],
                                      wproj_d[ic * 128:(ic + 1) * 128, :])
                    wpts.append(wpt)
                att["bk"] = bk_sb
                att["bp"] = bp_sb
                att["wk"] = wkts
                att["wp"] = wpts

            def layer(l, hooks):
                seqA = [(r, 0) for r in range(RSLOT)]
                seqB = [(r, 1) for r in range(RSLOT)]
                seq = []
                ia = ib = 0
                while ia < len(seqA) or ib < len(seqB):
                    if ia < len(seqA) and (ia - ib < LEAD or ib >= len(seqB)):
                        seq.append(seqA[ia])
                        ia += 1
                    else:
                        seq.append(seqB[ib])
                        ib += 1
                last_a = (l, RSLOT - 1, 0)
                for k, (r, j) in enumerate(seq):
                    if k in hooks:
                        hooks[k]()
                    if combo(l, r, j) == last_a:
                        ship_half(l, 0)   # A done early: overlap its staging
                done = flush()
                if done == last_a:
                    ship_half(l, 0)
                ship(l)

            layer(0, {16: q_loads, 27: q_compute})
            h2_prep(0, 0)
            layer(1, {1: lambda: h2_prep(0, 1), 6: _load_att_weights})

            # =================== attention (own batch via mask) ==========
            qwT = qst["qwT"]
            r0 = miscp.tile([S, FE], dt.bfloat16, name="r0", tag="raw", bufs=2)
            nc.sync.dma_start(r0[:], arout[1][0][:])
            r1 = miscp.tile([S, FE], dt.bfloat16, name="r1", tag="raw", bufs=2)
            nc.sync.dma_start(r1[:], arout[1][1][:])
            rawf = miscp.tile([S, FE], dt.float32, name="rawf", tag="rawf")
            nc.vector.tensor_scalar(rawf[:], r0[:], mask_sb[:, 0:1], None,
                                    OP.mult)
            nc.vector.scalar_tensor_tensor(rawf[:], r1[:], mask_sb[:, 1:2],
                                           rawf[:], OP.mult, OP.add)
            rd = miscp.tile([S, 1], dt.float32, name="rdf", tag="rd")
            nc.vector.reciprocal(rd[:], rawf[:, F:FE])
            hf = miscp.tile([S, F], dt.bfloat16, name="hf", tag="h2")
            nc.scalar.activation(hf[:], rawf[:, :F], AF.Relu, scale=rd[:])
            hfT = hTp.tile([128, IC * S], dt.bfloat16, name="hfT")
            for ic in range(IC):
                tp = ps_ld.tile([128, 128], dt.bfloat16, name=f"ftp{ic}",
                                tag="ld")
                nc.tensor.transpose(tp[:], hf[:, ic * 128:(ic + 1) * 128],
                                    ident_b[:])
                nc.scalar.copy(hfT[:, ic * S:(ic + 1) * S], tp[:])

            bk_sb, bp_sb = att["bk"], att["bp"]
            wkts, wpts = att["wk"], att["wp"]
            # kx = hf @ wk + bk   [S, 768]
            kx_ps = ps_hid.tile([S, F], dt.float32, name="kx_ps", tag="hid")
            for ic in range(IC):
                lhsT = hfT[:, ic * S:(ic + 1) * S]
                nc.tensor.matmul(kx_ps[:, 0:512], lhsT=lhsT,
                                 rhs=wkts[ic][:, 0:512],
                                 start=(ic == 0), stop=False)
                nc.tensor.matmul(kx_ps[:, 512:F], lhsT=lhsT,
                                 rhs=wkts[ic][:, 512:F],
                                 start=(ic == 0), stop=False)
            nc.tensor.matmul(kx_ps[:, 0:512], lhsT=ones_row[:],
                             rhs=bk_sb[:, 0:512], start=False, stop=True)
            nc.tensor.matmul(kx_ps[:, 512:F], lhsT=ones_row[:],
                             rhs=bk_sb[:, 512:F], start=False, stop=True)
            kx = miscp.tile([S, F], dt.bfloat16, name="kx", tag="kx")
            nc.scalar.copy(kx[:], kx_ps[:])
            # kxT per head directly: kxT_h = sum_ic wk[ic,h]^T @ hfT[ic]
            kxT = miscp.tile([HD, NH * S], dt.bfloat16, name="kxT", tag="kxT")
            for hh in range(NH):
                ktp = ps_intm.tile([HD, S], dt.float32, name=f"ktp{hh}",
                                   tag="intm")
                for ic in range(IC):
                    nc.tensor.matmul(
                        ktp[:],
                        lhsT=wkts[ic][:, hh * HD:(hh + 1) * HD],
                        rhs=hfT[:, ic * S:(ic + 1) * S],
                        start=(ic == 0), stop=False)
                nc.tensor.matmul(ktp[:],
                                 lhsT=bk_sb[:, hh * HD:(hh + 1) * HD],
                                 rhs=ones_row[:], start=False, stop=True)
                nc.scalar.copy(kxT[:, hh * S:(hh + 1) * S], ktp[:])

            # scoreT[:,h] = kx_h @ qwT_h    [128, 8]
            sc_ps = ps_intm.tile([S, NH, 4], dt.float32, name="sc_ps",
                                 tag="intm")
            for hh in range(NH):
                nc.tensor.matmul(sc_ps[:, hh, 0:1],
                                 lhsT=kxT[:, hh * S:(hh + 1) * S],
                                 rhs=qwT[:, hh:hh + 1], start=True,
                                 stop=True)
            sc_sb = miscp.tile([S, NH], dt.float32, name="sc_sb", tag="scb", bufs=3)
            nc.scalar.copy(sc_sb[:], sc_ps[:, :, 0])
            # score rows [8, 128]
            srow_ps = ps_ld.tile([NH, S], dt.float32, name="srow", tag="ld")
            nc.tensor.transpose(srow_ps[:], sc_sb[:], ident_f[:])
            negmax = miscp.tile([NH, 1], dt.float32, name="negmax", tag="sm", bufs=3)
            nc.vector.tensor_reduce(negmax[:], srow_ps[:],
                                    mybir.AxisListType.X, OP.max,
                                    negate=True)
            esc = miscp.tile([NH, S], dt.float32, name="esc", tag="esc", bufs=2)
            sumexp = miscp.tile([NH, 1], dt.float32, name="sumexp", tag="sm", bufs=3)
            nc.scalar.activation(esc[:], srow_ps[:], AF.Exp, bias=negmax[:],
                                 accum_out=sumexp[:])
            rsm = miscp.tile([NH, 1], dt.float32, name="rsm", tag="sm", bufs=3)
            nc.vector.reciprocal(rsm[:], sumexp[:])
            attn = miscp.tile([NH, S], dt.bfloat16, name="attn", tag="esc", bufs=2)
            nc.vector.tensor_scalar_mul(attn[:], esc[:], rsm[:])
            # attnT [128, 8]
            at_ps = ps_ld.tile([S, NH], dt.bfloat16, name="at_ps", tag="ld")
            nc.tensor.transpose(at_ps[:], attn[:], ident_b[:NH, :NH])
            attnT = miscp.tile([S, NH], dt.bfloat16, name="attnT", tag="scb", bufs=3)
            nc.scalar.copy(attnT[:], at_ps[:])
            # o[0, h*96:(h+1)*96] = attn_h @ kx_h  (bank-safe [1,8,128] tiles)
            o_psA = ps_intm.tile([1, 4, 128], dt.float32, name="o_psA",
                                 tag="intm")
            o_psB = ps_intm.tile([1, 4, 128], dt.float32, name="o_psB",
                                 tag="intm")
            for hh in range(NH):
                tgt = o_psA if hh < 4 else o_psB
                nc.tensor.matmul(tgt[:, hh % 4, :HD],
                                 lhsT=attnT[:, hh:hh + 1],
                                 rhs=kx[:, hh * HD:(hh + 1) * HD],
                                 start=True, stop=True)
            o_sb = miscp.tile([1, F], dt.bfloat16, name="o_sb", tag="qx")
            nc.scalar.copy(o_sb[:, 0:384], o_psA[:, :, :HD])
            nc.scalar.copy(o_sb[:, 384:F], o_psB[:, :, :HD])
            # oT [128, 6]
            oT_ps = ps_ld.tile([S, IC, 4], dt.bfloat16, name="oT_ps", tag="ld")
            for ic in range(IC):
                nc.tensor.transpose(oT_ps[:, ic, 0:1],
                                    o_sb[:, ic * 128:(ic + 1) * 128],
                                    ident_b[:1, :1])
            oT = miscp.tile([S, IC], dt.bfloat16, name="oT", tag="scb", bufs=3)
            nc.scalar.copy(oT[:], oT_ps[:, :, 0])
            # res = o @ wproj + bproj
            res_ps = ps_hid.tile([1, 512], dt.float32, name="res_ps",
                                 tag="hid")
            res_ps2 = ps_hid.tile([1, 256], dt.float32, name="res_ps2",
                                  tag="hid")
            for ic in range(IC):
                nc.tensor.matmul(res_ps[:], lhsT=oT[:, ic:ic + 1],
                                 rhs=wpts[ic][:, 0:512],
                                 start=(ic == 0), stop=False)
                nc.tensor.matmul(res_ps2[:], lhsT=oT[:, ic:ic + 1],
                                 rhs=wpts[ic][:, 512:F],
                                 start=(ic == 0), stop=False)
            nc.tensor.matmul(res_ps[:], lhsT=one_sb[:], rhs=bp_sb[:, 0:512],
                             start=False, stop=True)
            nc.tensor.matmul(res_ps2[:], lhsT=one_sb[:], rhs=bp_sb[:, 512:F],
                             start=False, stop=True)
            res_sb = miscp.tile([1, F], dt.float32, name="res_sb", tag="res")
            nc.scalar.copy(res_sb[:, 0:512], res_ps[:])
            nc.scalar.copy(res_sb[:, 512:F], res_ps2[:])
            nc.sync.dma_start(out_d[:], res_sb[:])

    nc.compile()
    _CACHE["nc"] = nc
    return nc


def _prep_inputs(x, adj, q, w_rgcn, score_w, score_b, wk, bk, wq, bq, wbil,
                 wproj, bproj):
    f32 = np.float32
    x = np.asarray(x, f32)
    adj = np.asarray(adj, f32)
    q = np.asarray(q, f32)
    w_rgcn = np.asarray(w_rgcn, f32)
    score_w = np.asarray(score_w, f32)
    score_b = np.asarray(score_b, f32)

    u = np.einsum("lrio,lo->lri", w_rgcn, score_w).astype(f32)
    w_ext = np.concatenate([w_rgcn, u[..., None]], axis=-1)  # [2,41,768,769]
    # pre-permute: [l, r, i, f] -> [l, r, p, (c f)] with i = c*128 + p
    w_perm = np.ascontiguousarray(
        w_ext.reshape(NL, R, IC, 128, FE).transpose(0, 1, 3, 2, 4)
        .reshape(NL, R, 128, IC * FE)).astype(bf16)

    # adj rowsum reciprocal (device never computes denominators)
    den = adj.sum(axis=3)                                    # [B, R, S]
    rec = (1.0 / np.where(den == 0.0, 1.0, den)).astype(f32)

    shared = {
        "wk": np.asarray(wk, f32).astype(bf16),
        "wq": np.asarray(wq, f32).astype(bf16),
        "wbil": np.asarray(wbil, f32).astype(bf16),
        "wproj": np.asarray(wproj, f32).astype(bf16),
        "bk": np.asarray(bk, f32).reshape(1, F).astype(bf16),
        "bq": np.asarray(bq, f32).reshape(1, F).astype(bf16),
        "bproj": np.asarray(bproj, f32).reshape(1, F).astype(bf16),
    }

    in_maps = []
    for c in range(NCORES):
        p, h = c // 2, c % 2
        A, Bb = 2 * p, 2 * p + 1
        rsel = list(range(0, 20) if h == 0 else range(20, 40)) + [R - 1]
        w_c = w_perm[:, rsel]                                # [2,21,128,IC*FE]
        adjt_c = np.empty((RSLOT, 128, 2 * S), f32)
        rec_c = np.empty((S, 2 * RSLOT), f32)
        for j, bb in enumerate((A, Bb)):
            adjt_c[:, :, j * S:(j + 1) * S] = adj[bb, rsel].transpose(0, 2, 1)
            rec_c[:, 2 * np.arange(RSLOT) + j] = rec[bb, rsel].T
        eb_c = np.empty((S, 2 * NL), f32)
        for l in range(NL):
            eb_c[:, 2 * l] = score_b[l]
            eb_c[:, 2 * l + 1] = score_b[l] + np.log(0.5)
        mask_c = np.zeros((S, 2), f32)
        mask_c[:, h] = 1.0
        xh = np.empty((2, 128, IC * S), f32)
        for j, bb in enumerate((A, Bb)):
            xh[j] = (x[bb].T.reshape(IC, 128, S).transpose(1, 0, 2)
                     .reshape(128, IC * S))
        m = dict(shared)
        m["w"] = np.ascontiguousarray(w_c)
        m["adjt"] = np.ascontiguousarray(adjt_c).astype(bf16)
        m["rec"] = rec_c
        m["ebias"] = eb_c
        m["mask"] = mask_c
        m["qcol"] = q[c].reshape(IC, S, 1).astype(bf16)
        m["xt2"] = xh.astype(bf16)
        in_maps.append(m)
    return in_maps


def kernel(**inputs) -> np.ndarray:
    from concourse.bass_utils import run_bass_kernel_spmd

    nc = _build_graph()
    in_maps = _prep_inputs(**inputs)
    res = run_bass_kernel_spmd(nc, in_maps, core_ids=list(range(NCORES)))
    outs = [np.asarray(res.results[c]["out"], np.float32) for c in range(NCORES)]
    return np.stack(outs)  # [8, 1, 768]
